# revision 11
# baseline (speedup 1.0000x reference)
"""AdaptiveLocalPooling Trainium2 kernel (8 NeuronCores, batch-sharded).

For each (b, t): gather K=9 neighbor rows X[b, idx[t,k], :], cosine-sim
against X[b, t, :], softmax over K, weighted-pool the neighbors, then mean
over t -> cls [B, 1, C].

Per-core plan (B_local=2, T=4096, C=384, K=9):
  1. Pre-pass (bulk): gpsimd cast-DMAs X f32 -> bf16 into a resident SBUF
     table tab_sb [128, 32*768] (partition p holds rows t%128==p,
     tile-major, both batches packed). Sync stores each chunk's x-part to
     the DRAM gather table (1792B rows) as it lands. DVE computes row
     sq-norms from tab_sb in bulk; one reciprocal + ACT sqrt produce
     resident f32 inv-norms (query side) and a bf16 pair per row that a
     single strided DMA drops into the DRAM rows' tail (cols 768:770);
     row pad bytes (770:896) are never read.
  2. Main loop over 32 tiles of 128 t's:
       - the 9*128 neighbor rows (1792B each, incl. the bf16 inv-norm
         pair) are gathered with TWO dma_gather calls per tile, queue
         pair alternating across tiles over 4 SWDGE queues: 4 active
         descriptor rings keep all 16 SDMA engines busy (a single ring
         tops out ~206 GB/s; 4 rings reach the ~350 GB/s HBM roofline).
         Calls must stay <= 1024 idxs (bigger hangs the ring).
       - queries are read DIRECTLY from tab_sb (no per-tile DMA).
       - dot[p,k,b] via fused scalar_tensor_tensor (mult+mult,
         accum_out) with the query inv-norm folded into the
         per-partition scalar. DVE runs at ~1 elem/cycle/lane on
         2-stream ops, so the 18 dot STTs are SPLIT: k=0..5 on DVE,
         k=6..8 on GPSIMD (engine-serial with its own gather-gen, two
         tiles behind so it never waits on its own gathers).
       - sim2 = dots * gathered-ninv (strided TT); softmax: ACT Exp
         with fused per-batch accum_out denominators; DVE reciprocal;
         w = e * sinv (bf16). All reduction-class outputs (STT/ACT
         accum_out, reciprocal) land LATE on HW and are read >= ~19
         instructions after production (lag-3/4 phase pipeline).
       - pooled+mean: 18 PE matmuls [1x384] per tile accumulating
         sum_t sum_k w * X[idx] into PSUM across the whole kernel;
         epilogue scales by 1/T.
"""

import os
import sys

import numpy as np

for _p in ("/opt/trn_rl_repo", "/root/.axon_site/_ro/trn_rl_repo"):
    if os.path.isdir(_p) and _p not in sys.path:
        sys.path.insert(0, _p)

import concourse.bacc as bacc
import concourse.bass as bass
import concourse.mybir as mybir
from concourse.bass_utils import run_bass_kernel_spmd
from concourse.library_config import mlp

# Problem sizes (hardcoded per spec).
B = 16
T = 4096
C = 384
K = 9
N_CORES = 8
B_LOC = B // N_CORES  # 2

P = 128
NT = T // P  # 32 tiles
XCOL = 2 * C  # 768 x-payload elems per table row
ROW_PAD = 896  # DRAM row stride in elems (1792 B, divisible by 256)
NIDX = K * P  # 1152 gathered rows per tile
IDX_COLS = NIDX // 16  # 72 idx columns per tile in the wrapped layout
NCHUNK = 4  # pre-pass chunks
TPC = NT // NCHUNK  # tiles per chunk
GDEPTH = 8  # G buffer depth
NQ = 4  # SWDGE queues for the gather
SPANS = ((0, 5), (5, 9))  # row-group split across the tile's 2 calls
KV = K  # dot STTs k<KV on DVE, k>=KV on gpsimd (Pool lacks STT codegen)
GLAG = 2  # gpsimd dot STTs run GLAG tiles behind its gather issue

FP32 = mybir.dt.float32
BF16 = mybir.dt.bfloat16
I16 = mybir.dt.int16
AX = mybir.AxisListType
OP = mybir.AluOpType
AF = mybir.ActivationFunctionType


def build_kernel(
    n_tiles: int = NT,
    debug: bool = False,
    pre_reps: int = 1,
    main_reps: int = 1,
    ablate: str = "",
) -> bass.Bass:
    # ablate: comma list of {stt, gather, pe} to stub out (timing-only builds)
    t_loc = n_tiles * P
    nch = max(1, n_tiles // TPC)
    nc = bacc.Bacc("TRN2", num_swdge_queues=NQ)

    x_ext = nc.declare_dram_parameter("X", [B_LOC, t_loc, C], FP32, isOutput=False)
    idx_ext = nc.declare_dram_parameter(
        "idx", [P, n_tiles * IDX_COLS], I16, isOutput=False
    )
    out_ext = nc.declare_dram_parameter("out", [B_LOC, C], FP32, isOutput=True)
    table = nc.dram_tensor("table", [t_loc, ROW_PAD], BF16)

    from contextlib import ExitStack

    with ExitStack() as ctx:
        e = ctx.enter_context

        idx_sb = e(nc.sbuf_tensor("idx_sb", [P, n_tiles * IDX_COLS], I16))
        tab_sb = e(nc.sbuf_tensor("tab_sb", [P, n_tiles * XCOL], BF16))
        nn_all = e(nc.sbuf_tensor("nn_all", [P, 2 * n_tiles], FP32))
        nrec_all = e(nc.sbuf_tensor("nrec_all", [P, 2 * n_tiles], FP32))
        ninv_sb = e(nc.sbuf_tensor("ninv_sb", [P, 2 * n_tiles], FP32))
        ninv_bf = e(nc.sbuf_tensor("ninv_bf", [P, 2 * n_tiles], BF16))
        # main-loop buffers
        G = [
            e(nc.sbuf_tensor(f"G{i}", [P, K * ROW_PAD], BF16)) for i in range(GDEPTH)
        ]
        scr = e(nc.sbuf_tensor("scr", [P, C], BF16))
        scr2 = e(nc.sbuf_tensor("scr2", [P, C], BF16))
        dots = [e(nc.sbuf_tensor(f"dots{i}", [P, 2 * K], FP32)) for i in range(4)]
        sim2 = [e(nc.sbuf_tensor(f"sim2_{i}", [P, 2 * K], FP32)) for i in range(2)]
        ee = [e(nc.sbuf_tensor(f"e{i}", [P, 2 * K], FP32)) for i in range(2)]
        ss = [e(nc.sbuf_tensor(f"s{i}_", [P, 2], FP32)) for i in range(2)]
        sinv = e(nc.sbuf_tensor("sinv", [P, 2], FP32))
        dscr = e(nc.sbuf_tensor("dscr", [P, 2], FP32))
        ww = [e(nc.sbuf_tensor(f"w{i}", [P, 2 * K], BF16)) for i in range(4)]
        out_sb = e(nc.sbuf_tensor("out_sb", [33, C], FP32))
        acc = e(nc.psum_tensor("acc", [64, C], FP32))

        xload = e(nc.semaphore("xload"))  # +32 per cast chunk (2 DMAs)
        vchk = e(nc.semaphore("vchk"))  # +1 per DVE STT chunk
        vpre = e(nc.semaphore("vpre"))  # +1 per pre-pass rep (recip done)
        spre = e(nc.semaphore("spre"))  # +1 per pre-pass rep (ACT done)
        tdone = e(nc.semaphore("tdone"))  # +16 per table store
        isem = e(nc.semaphore("isem"))
        gsem = [e(nc.semaphore(f"gsem{i}")) for i in range(GDEPTH)]
        gdot = e(nc.semaphore("gdot"))  # gpsimd dot STTs done, +1/tile
        v1 = e(nc.semaphore("v1"))  # DVE sim2 done, +1/tile
        v2 = e(nc.semaphore("v2"))  # DVE ww done, +1/tile
        aexp = e(nc.semaphore("aexp"))  # ACT exp done, +1/tile
        pe_done = e(nc.semaphore("pe_done"))
        vfin = e(nc.semaphore("vfin"))
        osem = e(nc.semaphore("osem"))

        block = e(nc.Block())
        n_main = main_reps * n_tiles
        stores_per_rep = nch + 1  # nch x-part chunks + 1 norm-tail store

        def tabx(t, b):  # query x slice of tab_sb for tile t, batch b
            return tab_sb[:, t * XCOL + b * C : t * XCOL + (b + 1) * C]

        def k3(ap):  # [P, 2K] -> [P, K, 2] (k-major pairs)
            return ap.rearrange("p (a b) -> p a b", b=2)

        def bcast2(ap2):  # [P, 2] -> [P, K, 2] with step-0 broadcast over K
            return ap2.rearrange("p (o b) -> p o b", o=1).to_broadcast([P, K, 2])

        def dot_stt(eng, g, k, b, out_scr):
            # query inv-norm is folded into the ACT exp scale, not here
            return eng.scalar_tensor_tensor(
                out=out_scr[:],
                in0=tabx(g % n_tiles, b),
                scalar=1.0,
                in1=G[g % GDEPTH][
                    :, k * ROW_PAD + b * C : k * ROW_PAD + (b + 1) * C
                ],
                op0=OP.mult,
                op1=OP.mult,
                accum_out=dots[g % 4][:, k * 2 + b : k * 2 + b + 1],
            )

        @block.sync
        def _(sync: bass.BassEngine):
            # ---- pre-pass: store x-part chunks + the norm tail ----
            for r in range(pre_reps):
                for c in range(nch):
                    sync.wait_ge(xload, 32 * (r * nch + c + 1))
                    rows = slice(c * TPC * P, (c + 1) * TPC * P)
                    sync.dma_start(
                        out=table[rows, 0:XCOL].rearrange("(g p) r -> p g r", p=P),
                        in_=tab_sb[
                            :, c * TPC * XCOL : (c + 1) * TPC * XCOL
                        ].rearrange("p (g r) -> p g r", r=XCOL),
                    ).then_inc(tdone, 16)
                sync.wait_ge(spre, r + 1)
                sync.dma_start(
                    out=table[:, XCOL : XCOL + 2].rearrange(
                        "(g p) r -> p g r", p=P
                    ),
                    in_=ninv_bf[:].rearrange("p (g b) -> p g b", b=2),
                ).then_inc(tdone, 16)
            # ---- epilogue ----
            sync.wait_ge(vfin, 1)
            sync.dma_start(out=out_ext[0:1, :], in_=out_sb[0:1, :]).then_inc(osem, 16)
            sync.dma_start(out=out_ext[1:2, :], in_=out_sb[32:33, :]).then_inc(
                osem, 16
            )
            sync.wait_ge(osem, 32)

        @block.vector
        def _(vector: bass.BassVectorEngine):
            # ---- pre-pass: bulk query norms from tab_sb ----
            for r in range(pre_reps):
                for c in range(nch):
                    vector.wait_ge(xload, 32 * (r * nch + c + 1))
                    for tt in range(TPC):
                        t = c * TPC + tt
                        for b in range(2):
                            st = vector.scalar_tensor_tensor(
                                out=scr[:],
                                in0=tabx(t, b),
                                scalar=1.0,
                                in1=tabx(t, b),
                                op0=OP.mult,
                                op1=OP.mult,
                                accum_out=nn_all[:, 2 * t + b : 2 * t + b + 1],
                            )
                    st.then_inc(vchk, 1)
                # spacers so the last chunk's accum lands before the recip
                for _ in range(3):
                    vector.tensor_copy(out=scr[:], in_=tab_sb[:, 0:C])
                vector.reciprocal(out=nrec_all[:], in_=nn_all[:]).then_inc(vpre, 1)

            # ---- main loop (see module docstring for the phase schedule) ----
            def phase_a1(m):  # sinv = 1/ss for tile m (ss from ACT accum)
                vector.wait_ge(aexp, m + 1)
                vector.reciprocal(out=sinv[:], in_=ss[m % 2][:])

            def phase_b(m):  # sim2 = dots * gathered-ninv for tile m
                if KV < K:
                    vector.wait_ge(gdot, m + 1)  # gpsimd's k>=KV dots landed
                gv = G[m % GDEPTH][:].rearrange("p (g r) -> p g r", r=ROW_PAD)
                vector.tensor_tensor(
                    out=k3(sim2[m % 2][:]),
                    in0=k3(dots[m % 4][:]),
                    in1=gv[:, :, XCOL : XCOL + 2],
                    op=OP.mult,
                ).then_inc(v1, 1)

            def phase_a2(m):  # ww = ee * sinv for tile m
                if m >= 4:
                    vector.wait_ge(pe_done, m - 3)  # ww[m%4] free
                vector.tensor_tensor(
                    out=k3(ww[m % 4][:]),
                    in0=k3(ee[m % 2][:]),
                    in1=bcast2(sinv[:]),
                    op=OP.mult,
                ).then_inc(v2, 1)

            def spacer(n=2):
                for _ in range(n):
                    vector.tensor_copy(out=scr[:], in_=tab_sb[:, 0:C])

            nspans = len(SPANS)
            vector.wait_ge(spre, pre_reps)  # ninv_sb resident
            for g in range(n_main):
                if g >= 3:
                    phase_b(g - 3)
                if g >= 4:
                    phase_a1(g - 4)
                vector.wait_ge(gsem[g % GDEPTH], 16 * nspans * (g // GDEPTH + 1))
                for k in range(1 if "stt" in ablate else KV):
                    for b in range(2):
                        dot_stt(vector, g, k, b, scr)
                if g >= 4:
                    phase_a2(g - 4)
            # tail (in-loop: b m<=n-4, a1/a2 m<=n-5)
            nm = n_main
            spacer(3)
            phase_b(nm - 3)
            phase_a1(nm - 4)
            spacer(3)
            phase_a2(nm - 4)
            phase_b(nm - 2)
            phase_a1(nm - 3)
            spacer(3)
            phase_a2(nm - 3)
            phase_b(nm - 1)
            phase_a1(nm - 2)
            spacer(3)
            phase_a2(nm - 2)
            phase_a1(nm - 1)
            spacer(3)
            phase_a2(nm - 1)
            # epilogue: PSUM -> SBUF with 1/T scaling
            vector.wait_ge(pe_done, n_main)
            vector.tensor_scalar_mul(
                out=out_sb[0:1, :], in0=acc[0:1, :], scalar1=1.0 / (t_loc * main_reps)
            )
            vector.tensor_scalar_mul(
                out=out_sb[32:33, :],
                in0=acc[32:33, :],
                scalar1=1.0 / (t_loc * main_reps),
            ).then_inc(vfin, 1)

        @block.scalar
        def _(scalar: bass.BassScalarEngine):
            # ---- pre-pass: ninv = sqrt(1/nn) -> resident f32 + bf16 copy ----
            for r in range(pre_reps):
                scalar.wait_ge(vpre, r + 1)
                scalar.activation(out=ninv_sb[:], in_=nrec_all[:], func=AF.Sqrt)
                scalar.activation(
                    out=ninv_bf[:], in_=ninv_sb[:], func=AF.Copy
                ).then_inc(spre, 1)
            # ---- main loop: exp with fused per-batch denominators; the
            # aexp inc rides a trailing dummy op so the accum lands first ----
            for m in range(n_main):
                scalar.wait_ge(v1, m + 1)
                if m >= 2:
                    scalar.wait_ge(v2, m - 1)  # ee/ss[m%2] free (a2 lag 4)
                s23 = k3(sim2[m % 2][:])
                e3 = k3(ee[m % 2][:])
                tq = m % n_tiles
                for b in range(2):
                    scalar.activation(
                        out=e3[:, :, b : b + 1],
                        in_=s23[:, :, b : b + 1],
                        func=AF.Exp,
                        scale=ninv_sb[:, 2 * tq + b : 2 * tq + b + 1],
                        accum_out=ss[m % 2][:, b : b + 1],
                    )
                scalar.activation(
                    out=dscr[:], in_=sim2[m % 2][:, 0:2], func=AF.Copy
                ).then_inc(aexp, 1)

        @block.tensor
        def _(tensor: bass.BassTensorEngine):
            for g in range(n_main):
                tensor.wait_ge(v2, g + 1)
                for k in range(1 if "pe" in ablate else K):
                    for b in range(2):
                        mm = tensor.matmul(
                            out=acc[32 * b : 32 * b + 1, :],
                            lhsT=ww[g % 4][:, k * 2 + b : k * 2 + b + 1],
                            rhs=G[g % GDEPTH][
                                :, k * ROW_PAD + b * C : k * ROW_PAD + (b + 1) * C
                            ],
                            start=(g == 0 and k == 0),
                            stop=(g == n_main - 1 and k == K - 1),
                            skip_group_check=True,
                        )
                mm.then_inc(pe_done, 1)

        @block.gpsimd
        def _(gpsimd: bass.BassGpSimd):
            gpsimd.load_library(mlp)
            gpsimd.dma_start(out=idx_sb[:], in_=idx_ext[:]).then_inc(isem, 16)
            # ---- pre-pass: cast-DMA X f32 -> tab_sb bf16, chunked ----
            for r in range(pre_reps):
                for c in range(nch):
                    if r > 0:
                        # previous rep's consumers of this chunk must finish
                        gpsimd.wait_ge(vchk, (r - 1) * nch + c + 1)
                        gpsimd.wait_ge(
                            tdone, 16 * ((r - 1) * stores_per_rep + c + 1)
                        )
                    rows = slice(c * TPC * P, (c + 1) * TPC * P)
                    tv = tab_sb[
                        :, c * TPC * XCOL : (c + 1) * TPC * XCOL
                    ].rearrange("p (g b c) -> p g b c", b=2, c=C)
                    for b in range(2):
                        gpsimd.dma_start(
                            out=tv[:, :, b, :],
                            in_=x_ext[b, rows, :].rearrange(
                                "(g p) c -> p g c", p=P
                            ),
                        ).then_inc(xload, 16)
            # ---- main loop: gathers + the k>=KV dot STTs (GLAG behind) ----
            gpsimd.wait_ge(isem, 16)
            gpsimd.wait_ge(tdone, 16 * pre_reps * stores_per_rep)
            nspans = len(SPANS)

            def gp_dots(m):
                gpsimd.wait_ge(gsem[m % GDEPTH], 16 * nspans * (m // GDEPTH + 1))
                for k in range(KV, K):
                    for b in range(2):
                        st = dot_stt(gpsimd, m, k, b, scr2)
                st.then_inc(gdot, 1)

            for g in range(n_main):
                t = g % n_tiles
                if g >= GDEPTH:
                    gpsimd.wait_ge(pe_done, g - (GDEPTH - 2))  # G[g%GDEPTH] free
                gv3 = G[g % GDEPTH][:].rearrange("p (g r) -> p g r", r=ROW_PAD)
                spans = ((0, 1), (1, 2)) if "gather" in ablate else SPANS
                for si, (g0, g1) in enumerate(spans):
                    n = (g1 - g0) * P
                    gpsimd.dma_gather(
                        gv3[:, g0:g1, :],
                        table[:],
                        idx_sb[
                            :,
                            t * IDX_COLS + g0 * 8 : t * IDX_COLS + g1 * 8,
                        ],
                        n,
                        n,
                        ROW_PAD,
                        single_packet=True,
                        queue_num=(2 * g + si) % NQ,
                    ).then_inc(gsem[g % GDEPTH], 16)
                if KV < K and "stt" not in ablate and g >= GLAG:
                    gp_dots(g - GLAG)
            if KV < K:
                if "stt" not in ablate:
                    for m in range(max(0, n_main - GLAG), n_main):
                        gp_dots(m)
                else:
                    for m in range(n_main):
                        gpsimd.tensor_copy(
                            out=scr2[:, 0:2], in_=scr2[:, 0:2]
                        ).then_inc(gdot, 1)

    nc.compile()
    return nc


def make_idx_table(neighbor_idx: np.ndarray, n_tiles: int = NT) -> np.ndarray:
    """Host-side index preprocessing into dma_gather's wrapped int16 layout.

    Flat order per tile: i = k*128 + p  ->  neighbor_idx[t0+p, k].
    Wrapped: idx_sb[q, tile*IDX_COLS + c] = flat[c*16 + q%16].
    """
    nb = np.asarray(neighbor_idx).astype(np.int16)  # values < 4096
    cols = np.empty((P, n_tiles * IDX_COLS), dtype=np.int16)
    for t in range(n_tiles):
        flat = nb[t * P : (t + 1) * P, :].T.reshape(-1)  # [K*P], k-major
        wrap = flat.reshape(IDX_COLS, 16).T  # [16, IDX_COLS]
        cols[:, t * IDX_COLS : (t + 1) * IDX_COLS] = np.tile(wrap, (8, 1))
    return cols


_NC_CACHE: dict = {}


def _get_nc():
    if "nc" not in _NC_CACHE:
        _NC_CACHE["nc"] = build_kernel()
    return _NC_CACHE["nc"]


def kernel(X: np.ndarray, neighbor_idx: np.ndarray, **_ignored) -> np.ndarray:
    X = np.asarray(X, dtype=np.float32)
    idx_cols = make_idx_table(neighbor_idx)
    nc = _get_nc()
    core_ids = list(range(N_CORES))
    in_maps = [
        {"X": np.ascontiguousarray(X[i * B_LOC : (i + 1) * B_LOC]), "idx": idx_cols}
        for i in core_ids
    ]
    res = run_bass_kernel_spmd(nc, in_maps, core_ids)
    outs = [res.results[i]["out"] for i in range(N_CORES)]  # each [B_LOC, C]
    full = np.concatenate(outs, axis=0).reshape(B, 1, C).astype(np.float32)
    return full


if __name__ == "__main__":
    rng = np.random.default_rng(0)
    X = rng.standard_normal((B, T, C), dtype=np.float32)
    nb = rng.integers(0, T, size=(T, K)).astype(np.int64)
    out = kernel(X, nb)
    print("out", out.shape, out.dtype, float(np.abs(out).mean()))


# revision 12
# speedup vs baseline: 2.3474x; 2.3474x over previous
"""AdaptiveLocalPooling Trainium2 kernel (8 NeuronCores, batch-sharded).

For each (b, t): gather K=9 neighbor rows X[b, idx[t,k], :], cosine-sim
against X[b, t, :], softmax over K, weighted-pool the neighbors, then mean
over t -> cls [B, 1, C].

Per-core plan (B_local=2, T=4096, C=384, K=9):
  1. Pre-pass: build a bf16 "gather table" in DRAM: row j =
     [X[b0,j,:] bf16 | X[b1,j,:] bf16 | invnorm_b0 | invnorm_b1 | pad]
     (896 elems = 1792 B). Row norms are computed in f32 on DVE
     (tensor_tensor_reduce), inverted (DVE reciprocal + ACT sqrt), and also
     kept resident in SBUF for the query side.
  2. Main loop over 32 tiles of 128 t's:
       - gpsimd.dma_gather pulls the 9*128 neighbor rows (one 1792B
         descriptor covers both batches AND their inv-norms); 8-deep
         G/Q buffering keeps the SDMA gather stream running ahead of
         compute (the gather is the byte-bound bottleneck, ~145 GB/s
         effective on random 1792B reads).
       - queries come in via a plain sequential DMA of table rows.
       - dot[p,k,b] via fused scalar_tensor_tensor (mult+mult, accum_out),
         with the query inv-norm folded into the per-partition scalar.
       - sim2 = dots * gathered-ninv (strided TT); softmax: ACT Exp with
         fused per-batch accum_out denominators; DVE reciprocal;
         w = e * sinv (bf16).  All reduction-class outputs (STT/ACT
         accum_out, reciprocal) land LATE on HW and are only read >= ~19
         instructions after production (lag-3 phase pipeline).
       - pooled+mean: 18 PE matmuls [1x384] per tile accumulating
         sum_t sum_k w * X[idx] directly into PSUM across the whole
         kernel; epilogue scales by 1/T.
"""

import os
import sys

import numpy as np

for _p in ("/opt/trn_rl_repo", "/root/.axon_site/_ro/trn_rl_repo"):
    if os.path.isdir(_p) and _p not in sys.path:
        sys.path.insert(0, _p)

import concourse.bacc as bacc
import concourse.bass as bass
import concourse.mybir as mybir
from concourse.bass_utils import run_bass_kernel_spmd
from concourse.library_config import mlp

# Problem sizes (hardcoded per spec).
B = 16
T = 4096
C = 384
K = 9
N_CORES = 8
B_LOC = B // N_CORES  # 2

P = 128
NT = T // P  # 32 tiles
ROW = 2 * C + 2  # 770 payload elems per table row
ROW_PAD = 896  # padded to 1792 B (divisible by 256)
NIDX = K * P  # 1152 gathered rows per tile
IDX_COLS = NIDX // 16  # 72 idx columns per tile in the wrapped layout

FP32 = mybir.dt.float32
BF16 = mybir.dt.bfloat16
I16 = mybir.dt.int16
AX = mybir.AxisListType
OP = mybir.AluOpType
AF = mybir.ActivationFunctionType


def build_kernel(
    n_tiles: int = NT,
    debug: bool = False,
    pre_reps: int = 1,
    main_reps: int = 1,
    ablate: str = "",
) -> bass.Bass:
    # ablate: comma list of {stt, gather, pe} to stub out (timing-only builds)
    t_loc = n_tiles * P
    nc = bacc.Bacc("TRN2")

    x_ext = nc.declare_dram_parameter("X", [B_LOC, t_loc, C], FP32, isOutput=False)
    idx_ext = nc.declare_dram_parameter(
        "idx", [P, n_tiles * IDX_COLS], I16, isOutput=False
    )
    out_ext = nc.declare_dram_parameter("out", [B_LOC, C], FP32, isOutput=True)
    table = nc.dram_tensor("table", [t_loc, ROW_PAD], BF16)
    dbg = {}
    if debug:
        for nm, shape, dt in [
            ("d_ninv", [P, 2 * n_tiles], FP32),
            ("d_dots", [P, 2 * K], FP32),
            ("d_nvf", [P, 2 * K], FP32),
            ("d_sim2_0", [P, 2 * K], FP32),
            ("d_sim2_1", [P, 2 * K], FP32),
            ("d_e0", [P, 2 * K], FP32),
            ("d_e1", [P, 2 * K], FP32),
            ("d_s0", [P, 2], FP32),
            ("d_s1", [P, 2], FP32),
            ("d_w0", [P, 2 * K], BF16),
            ("d_w1", [P, 2 * K], BF16),
            ("d_q0", [P, ROW_PAD], BF16),
            ("d_q1", [P, ROW_PAD], BF16),
            ("d_g0", [P, K * ROW_PAD], BF16),
            ("d_g1", [P, K * ROW_PAD], BF16),
        ]:
            dbg[nm] = nc.declare_dram_parameter(nm, shape, dt, isOutput=True)

    from contextlib import ExitStack

    with ExitStack() as ctx:
        e = ctx.enter_context

        idx_sb = e(nc.sbuf_tensor("idx_sb", [P, n_tiles * IDX_COLS], I16))
        # pre-pass: 2-chunk rotating bf16 staging + bulk norm buffers
        XCOL = 2 * C
        TPC = 8
        nch = max(1, n_tiles // TPC)
        tstg = [
            e(nc.sbuf_tensor(f"tstg{i}", [P, TPC * XCOL], BF16)) for i in range(2)
        ]
        nn_all = e(nc.sbuf_tensor("nn_all", [P, 2 * n_tiles], FP32))
        nrec_all = e(nc.sbuf_tensor("nrec_all", [P, 2 * n_tiles], FP32))
        ninv_sb = e(nc.sbuf_tensor("ninv_sb", [P, 2 * n_tiles], FP32))
        ninv_bf = e(nc.sbuf_tensor("ninv_bf", [P, 2 * n_tiles], BF16))
        # main-loop buffers (G/Q x8 so gathers stream ahead; small ones x2)
        G = [e(nc.sbuf_tensor(f"G{i}", [P, K * ROW_PAD], BF16)) for i in range(8)]
        ghalf = (
            e(nc.sbuf_tensor("ghalf", [P, K * 512], BF16))
            if "halfbytes" in ablate
            else None
        )
        Q = [e(nc.sbuf_tensor(f"Q{i}", [P, ROW_PAD], BF16)) for i in range(8)]
        scr = e(nc.sbuf_tensor("scr", [P, C], BF16))
        dots = [e(nc.sbuf_tensor(f"dots{i}", [P, 2 * K], FP32)) for i in range(2)]
        sim2 = [e(nc.sbuf_tensor(f"sim2_{i}", [P, 2 * K], FP32)) for i in range(2)]
        ee = [e(nc.sbuf_tensor(f"e{i}", [P, 2 * K], FP32)) for i in range(2)]
        ss = [e(nc.sbuf_tensor(f"s{i}_", [P, 2], FP32)) for i in range(2)]
        sinv = e(nc.sbuf_tensor("sinv", [P, 2], FP32))
        dscr = e(nc.sbuf_tensor("dscr", [P, 2], FP32))
        ww = [e(nc.sbuf_tensor(f"w{i}", [P, 2 * K], BF16)) for i in range(4)]
        out_sb = e(nc.sbuf_tensor("out_sb", [33, C], FP32))
        acc = e(nc.psum_tensor("acc", [64, C], FP32))

        xload = e(nc.semaphore("xload"))
        vchk = e(nc.semaphore("vchk"))
        vpre = e(nc.semaphore("vpre"))
        spre = e(nc.semaphore("spre"))
        tdone = e(nc.semaphore("tdone"))
        isem = e(nc.semaphore("isem"))
        gsem = [e(nc.semaphore(f"gsem{i}")) for i in range(8)]
        qsem = [e(nc.semaphore(f"qsem{i}")) for i in range(8)]
        v1 = e(nc.semaphore("v1"))
        v2 = e(nc.semaphore("v2"))
        aexp = e(nc.semaphore("aexp"))
        pe_done = e(nc.semaphore("pe_done"))
        vfin = e(nc.semaphore("vfin"))
        osem = e(nc.semaphore("osem"))

        block = e(nc.Block())
        n_main = main_reps * n_tiles
        stores_per_rep = nch + 1
        n_stores = pre_reps * stores_per_rep

        def k3(ap):  # [P, 2K] -> [P, K, 2] (k-major pairs)
            return ap.rearrange("p (a b) -> p a b", b=2)

        def kT(ap):  # [P, 2K] -> [P, 2, 9] transposed view (reduce over k)
            return ap.rearrange("p (a b) -> p b a", b=2)

        def bcast2(ap2):  # [P, 2] -> [P, K, 2] with step-0 broadcast over K
            return ap2.rearrange("p (o b) -> p o b", o=1).to_broadcast([P, K, 2])

        @block.sync
        def _(sync: bass.BassEngine):
            # ---- pre-pass: store x-part chunks + the norm tail ----
            XC = XCOL
            for r in range(pre_reps):
                for c in range(nch):
                    sync.wait_ge(xload, 32 * (r * nch + c + 1))
                    rows = slice(c * TPC * P, (c + 1) * TPC * P)
                    sync.dma_start(
                        out=table[rows, 0:XC].rearrange("(g p) r -> p g r", p=P),
                        in_=tstg[c % 2][:].rearrange("p (g r) -> p g r", r=XC),
                    ).then_inc(tdone, 16)
                sync.wait_ge(spre, r + 1)
                sync.dma_start(
                    out=table[:, XC : XC + 2].rearrange("(g p) r -> p g r", p=P),
                    in_=ninv_bf[:].rearrange("p (g b) -> p g b", b=2),
                ).then_inc(tdone, 16)
            # ---- main loop: query loads (after full table resident) ----
            sync.wait_ge(tdone, 16 * n_stores)
            for g in range(n_main):
                t = g % n_tiles
                if g >= 8:
                    sync.wait_ge(v1, g - 6)  # Q[g%8] free (STTs g-7 done)
                sync.dma_start(
                    out=Q[g % 8][:, 0 : 2 * C],
                    in_=table[t * P : (t + 1) * P, 0 : 2 * C],
                ).then_inc(qsem[g % 8], 16)
            # ---- epilogue ----
            sync.wait_ge(vfin, 1)
            sync.dma_start(out=out_ext[0:1, :], in_=out_sb[0:1, :]).then_inc(osem, 16)
            sync.dma_start(out=out_ext[1:2, :], in_=out_sb[32:33, :]).then_inc(
                osem, 16
            )
            n_os = 32
            if debug:
                for nm, buf in []:
                    sync.dma_start(out=dbg[nm][:], in_=buf[:]).then_inc(osem, 16)
                    n_os += 16
            sync.wait_ge(osem, n_os)

        @block.vector
        def _(vector: bass.BassVectorEngine):
            # ---- pre-pass: bulk query norms from the staged bf16 chunks ----
            for r in range(pre_reps):
                for c in range(nch):
                    vector.wait_ge(xload, 32 * (r * nch + c + 1))
                    for tt in range(TPC):
                        t = c * TPC + tt
                        for b in range(2):
                            st = vector.scalar_tensor_tensor(
                                out=scr[:],
                                in0=tstg[c % 2][
                                    :, tt * XCOL + b * C : tt * XCOL + (b + 1) * C
                                ],
                                scalar=1.0,
                                in1=tstg[c % 2][
                                    :, tt * XCOL + b * C : tt * XCOL + (b + 1) * C
                                ],
                                op0=OP.mult,
                                op1=OP.mult,
                                accum_out=nn_all[:, 2 * t + b : 2 * t + b + 1],
                            )
                    st.then_inc(vchk, 1)
                for _ in range(3):
                    vector.tensor_copy(out=scr[:], in_=tstg[0][:, 0:C])
                vector.reciprocal(out=nrec_all[:], in_=nn_all[:]).then_inc(vpre, 1)

            # ---- main loop ----
            # iteration t: [A1] recip for t-2, [C] dots for t, [B] sim2 for
            # t-1, [A2] weights for t-2.  Reduction-class outputs (STT
            # accum_out, ACT accum_out, reciprocal) land late on HW, so every
            # such value is read >= ~19 instructions after it is produced.
            def phase_a1(m):  # sinv = 1/ss for tile m (ss from ACT accum)
                vector.wait_ge(aexp, m + 1)
                vector.reciprocal(out=sinv[:], in_=ss[m % 2][:])

            def phase_b(m):  # sim2 for tile m (reads dots[m%2], late-landing)
                gv = G[m % 8][:].rearrange("p (g r) -> p g r", r=ROW_PAD)
                vector.tensor_tensor(
                    out=k3(sim2[m % 2][:]),
                    in0=k3(dots[m % 2][:]),
                    in1=gv[:, :, 2 * C : 2 * C + 2],
                    op=OP.mult,
                ).then_inc(v1, 1)

            def phase_a2(m):  # ww = ee * sinv for tile m
                if m >= 4:
                    vector.wait_ge(pe_done, m - 3)  # ww[m%4] free
                vector.tensor_tensor(
                    out=k3(ww[m % 4][:]),
                    in0=k3(ee[m % 2][:]),
                    in1=bcast2(sinv[:]),
                    op=OP.mult,
                ).then_inc(v2, 1)

            def spacer(n=2):
                for _ in range(n):
                    vector.tensor_copy(out=scr[:], in_=Q[0][:, 0:C])

            vector.wait_ge(spre, pre_reps)  # ninv_sb resident
            for g in range(n_main):
                t = g % n_tiles
                if g >= 3:
                    phase_a1(g - 3)
                vector.wait_ge(gsem[g % 8], 32 * (g // 8 + 1))
                vector.wait_ge(qsem[g % 8], 16 * (g // 8 + 1))
                for k in range(1 if "stt" in ablate else K):
                    for b in range(2):
                        vector.scalar_tensor_tensor(
                            out=scr[:],
                            in0=Q[g % 8][:, b * C : (b + 1) * C],
                            scalar=ninv_sb[:, 2 * t + b : 2 * t + b + 1],
                            in1=G[g % 8][
                                :, k * ROW_PAD + b * C : k * ROW_PAD + (b + 1) * C
                            ],
                            op0=OP.mult,
                            op1=OP.mult,
                            accum_out=dots[g % 2][:, k * 2 + b : k * 2 + b + 1],
                        )
                if g >= 1:
                    phase_b(g - 1)
                if g >= 3:
                    phase_a2(g - 3)
            # tail (in-loop phases covered m <= n_main-4)
            for m in (n_main - 3, n_main - 2, n_main - 1):
                if m < 0 or m <= n_main - 4:
                    continue
                if m == n_main - 1:
                    spacer()
                    phase_b(m)
                phase_a1(m)
                spacer(3)
                phase_a2(m)
            # epilogue: PSUM -> SBUF with 1/T scaling
            vector.wait_ge(pe_done, n_main)
            vector.tensor_scalar_mul(
                out=out_sb[0:1, :], in0=acc[0:1, :], scalar1=1.0 / (t_loc * main_reps)
            )
            vector.tensor_scalar_mul(
                out=out_sb[32:33, :], in0=acc[32:33, :], scalar1=1.0 / (t_loc * main_reps)
            ).then_inc(vfin, 1)

        @block.scalar
        def _(scalar: bass.BassScalarEngine):
            # ---- pre-pass: ninv = sqrt(1/nn) -> resident f32 + bf16 copy ----
            for r in range(pre_reps):
                scalar.wait_ge(vpre, r + 1)
                scalar.activation(out=ninv_sb[:], in_=nrec_all[:], func=AF.Sqrt)
                scalar.activation(
                    out=ninv_bf[:], in_=ninv_sb[:], func=AF.Copy
                ).then_inc(spre, 1)
            # ---- main loop: exp with fused per-batch denominators; the
            # aexp inc rides a trailing dummy op so the accum lands first ----
            for g in range(n_main):
                scalar.wait_ge(v1, g + 1)
                if g >= 2:
                    scalar.wait_ge(v2, g - 1)  # ee/ss[g%2] free (a2 lag 3)
                s23 = k3(sim2[g % 2][:])
                e3 = k3(ee[g % 2][:])
                for b in range(2):
                    scalar.activation(
                        out=e3[:, :, b : b + 1],
                        in_=s23[:, :, b : b + 1],
                        func=AF.Exp,
                        accum_out=ss[g % 2][:, b : b + 1],
                    )
                scalar.activation(
                    out=dscr[:], in_=sim2[g % 2][:, 0:2], func=AF.Copy
                ).then_inc(aexp, 1)

        @block.tensor
        def _(tensor: bass.BassTensorEngine):
            for g in range(n_main):
                tensor.wait_ge(v2, g + 1)
                for k in range(1 if "pe" in ablate else K):
                    for b in range(2):
                        mm = tensor.matmul(
                            out=acc[32 * b : 32 * b + 1, :],
                            lhsT=ww[g % 4][:, k * 2 + b : k * 2 + b + 1],
                            rhs=G[g % 8][
                                :, k * ROW_PAD + b * C : k * ROW_PAD + (b + 1) * C
                            ],
                            start=(g == 0 and k == 0),
                            stop=(g == n_main - 1 and k == K - 1),
                            skip_group_check=True,
                        )
                mm.then_inc(pe_done, 1)

        @block.gpsimd
        def _(gpsimd: bass.BassGpSimd):
            gpsimd.load_library(mlp)
            gpsimd.dma_start(out=idx_sb[:], in_=idx_ext[:]).then_inc(isem, 16)
            # ---- pre-pass: cast-DMA X f32 -> staged bf16, chunked ----
            for r in range(pre_reps):
                for c in range(nch):
                    gi = r * nch + c  # global chunk index
                    if gi >= 2:
                        pr, pc = (gi - 2) // nch, (gi - 2) % nch
                        gpsimd.wait_ge(vchk, gi - 1)
                        gpsimd.wait_ge(tdone, 16 * (pr * stores_per_rep + pc + 1))
                    rows = slice(c * TPC * P, (c + 1) * TPC * P)
                    tv = tstg[c % 2][:].rearrange("p (g b c) -> p g b c", b=2, c=C)
                    for b in range(2):
                        gpsimd.dma_start(
                            out=tv[:, :, b, :],
                            in_=x_ext[b, rows, :].rearrange("(g p) c -> p g c", p=P),
                        ).then_inc(xload, 16)
            gpsimd.wait_ge(isem, 16)
            gpsimd.wait_ge(tdone, 16 * n_stores)  # full table resident
            for g in range(n_main):
                t = g % n_tiles
                if g >= 8:
                    gpsimd.wait_ge(pe_done, g - 7)  # G[g%8] free
                gv3 = G[g % 8][:].rearrange("p (g r) -> p g r", r=ROW_PAD)
                # 1024+128 split: the large first call keeps the SDMA
                # stream busier across call boundaries (~20% faster than 5+4)
                spans = ((0, 1), (1, 2)) if "gather" in ablate else ((0, 8), (8, K))
                sp_flag = True
                for g0, g1 in spans:
                    n = (g1 - g0) * P
                    if "halfbytes" in ablate:
                        gpsimd.dma_gather(
                            ghalf[:].rearrange("p (g r) -> p g r", r=512)[
                                :, g0:g1, :
                            ],
                            table[:, 0:512],
                            idx_sb[
                                :,
                                t * IDX_COLS + g0 * 8 : t * IDX_COLS + g1 * 8,
                            ],
                            n,
                            n,
                            512,
                            elem_step=ROW_PAD,
                        ).then_inc(gsem[g % 8], 16)
                    else:
                        gpsimd.dma_gather(
                            gv3[:, g0:g1, :],
                            table[:],
                            idx_sb[
                                :,
                                t * IDX_COLS + g0 * 8 : t * IDX_COLS + g1 * 8,
                            ],
                            n,
                            n,
                            ROW_PAD,
                            single_packet=sp_flag,
                        ).then_inc(gsem[g % 8], 16)

    nc.compile()
    return nc


def make_idx_table(neighbor_idx: np.ndarray, n_tiles: int = NT) -> np.ndarray:
    """Host-side index preprocessing into dma_gather's wrapped int16 layout.

    Flat order per tile: i = k*128 + p  ->  neighbor_idx[t0+p, k].
    Wrapped: idx_sb[q, tile*IDX_COLS + c] = flat[c*16 + q%16].
    """
    nb = np.asarray(neighbor_idx).astype(np.int16)  # values < 4096
    cols = np.empty((P, n_tiles * IDX_COLS), dtype=np.int16)
    for t in range(n_tiles):
        flat = nb[t * P : (t + 1) * P, :].T.reshape(-1)  # [K*P], k-major
        wrap = flat.reshape(IDX_COLS, 16).T  # [16, IDX_COLS]
        cols[:, t * IDX_COLS : (t + 1) * IDX_COLS] = np.tile(wrap, (8, 1))
    return cols


_NC_CACHE: dict = {}


def _get_nc():
    if "nc" not in _NC_CACHE:
        _NC_CACHE["nc"] = build_kernel()
    return _NC_CACHE["nc"]


def kernel(X: np.ndarray, neighbor_idx: np.ndarray, **_ignored) -> np.ndarray:
    X = np.asarray(X, dtype=np.float32)
    idx_cols = make_idx_table(neighbor_idx)
    nc = _get_nc()
    core_ids = list(range(N_CORES))
    in_maps = [
        {"X": np.ascontiguousarray(X[i * B_LOC : (i + 1) * B_LOC]), "idx": idx_cols}
        for i in core_ids
    ]
    res = run_bass_kernel_spmd(nc, in_maps, core_ids)
    outs = [res.results[i]["out"] for i in range(N_CORES)]  # each [B_LOC, C]
    full = np.concatenate(outs, axis=0).reshape(B, 1, C).astype(np.float32)
    return full


if __name__ == "__main__":
    rng = np.random.default_rng(0)
    X = rng.standard_normal((B, T, C), dtype=np.float32)
    nb = rng.integers(0, T, size=(T, K)).astype(np.int64)
    out = kernel(X, nb)
    print("out", out.shape, out.dtype, float(np.abs(out).mean()))



# revision 13
# speedup vs baseline: 2.5012x; 1.0656x over previous
"""AdaptiveLocalPooling Trainium2 kernel (8 NeuronCores, batch-sharded).

For each (b, t): gather K=9 neighbor rows X[b, idx[t,k], :], cosine-sim
against X[b, t, :], softmax over K, weighted-pool the neighbors, then mean
over t -> cls [B, 1, C].

Per-core plan (B_local=2, T=4096, C=384, K=9):
  1. Pre-pass: build a bf16 "gather table" in DRAM: row j =
     [X[b0,j,:] bf16 | X[b1,j,:] bf16 | invnorm_b0 | invnorm_b1 | pad]
     (896 elems = 1792 B). Row norms are computed in f32 on DVE
     (tensor_tensor_reduce), inverted (DVE reciprocal + ACT sqrt), and also
     kept resident in SBUF for the query side.
  2. Main loop over 32 tiles of 128 t's:
       - gpsimd.dma_gather pulls the 9*128 neighbor rows (one 1792B
         descriptor covers both batches AND their inv-norms); 8-deep
         G/Q buffering keeps the SDMA gather stream running ahead of
         compute (the gather is the byte-bound bottleneck, ~145 GB/s
         effective on random 1792B reads).
       - queries come in via a plain sequential DMA of table rows.
       - dot[p,k,b] via fused scalar_tensor_tensor (mult+mult, accum_out),
         with the query inv-norm folded into the per-partition scalar.
       - sim2 = dots * gathered-ninv (strided TT); softmax: ACT Exp with
         fused per-batch accum_out denominators; DVE reciprocal;
         w = e * sinv (bf16).  All reduction-class outputs (STT/ACT
         accum_out, reciprocal) land LATE on HW and are only read >= ~19
         instructions after production (lag-3 phase pipeline).
       - pooled+mean: 18 PE matmuls [1x384] per tile accumulating
         sum_t sum_k w * X[idx] directly into PSUM across the whole
         kernel; epilogue scales by 1/T.
"""

import os
import sys

import numpy as np

for _p in ("/opt/trn_rl_repo", "/root/.axon_site/_ro/trn_rl_repo"):
    if os.path.isdir(_p) and _p not in sys.path:
        sys.path.insert(0, _p)

import concourse.bacc as bacc
import concourse.bass as bass
import concourse.mybir as mybir
from concourse.bass_utils import run_bass_kernel_spmd
from concourse.library_config import mlp

# Problem sizes (hardcoded per spec).
B = 16
T = 4096
C = 384
K = 9
N_CORES = 8
B_LOC = B // N_CORES  # 2

P = 128
NT = T // P  # 32 tiles
ROW = 2 * C + 2  # 770 payload elems per table row
ROW_PAD = 896  # padded to 1792 B (divisible by 256)
NIDX = K * P  # 1152 gathered rows per tile
IDX_COLS = NIDX // 16  # 72 idx columns per tile in the wrapped layout

FP32 = mybir.dt.float32
BF16 = mybir.dt.bfloat16
I16 = mybir.dt.int16
AX = mybir.AxisListType
OP = mybir.AluOpType
AF = mybir.ActivationFunctionType


def build_kernel(
    n_tiles: int = NT,
    debug: bool = False,
    pre_reps: int = 1,
    main_reps: int = 1,
    ablate: str = "",
) -> bass.Bass:
    # ablate: comma list of {stt, gather, pe} to stub out (timing-only builds)
    t_loc = n_tiles * P
    nc = bacc.Bacc("TRN2")

    x_ext = nc.declare_dram_parameter("X", [B_LOC, t_loc, C], FP32, isOutput=False)
    idx_ext = nc.declare_dram_parameter(
        "idx", [P, n_tiles * IDX_COLS], I16, isOutput=False
    )
    out_ext = nc.declare_dram_parameter("out", [B_LOC, C], FP32, isOutput=True)
    table = nc.dram_tensor("table", [t_loc, ROW_PAD], BF16)
    dbg = {}
    if debug:
        for nm, shape, dt in [
            ("d_ninv", [P, 2 * n_tiles], FP32),
            ("d_dots", [P, 2 * K], FP32),
            ("d_nvf", [P, 2 * K], FP32),
            ("d_sim2_0", [P, 2 * K], FP32),
            ("d_sim2_1", [P, 2 * K], FP32),
            ("d_e0", [P, 2 * K], FP32),
            ("d_e1", [P, 2 * K], FP32),
            ("d_s0", [P, 2], FP32),
            ("d_s1", [P, 2], FP32),
            ("d_w0", [P, 2 * K], BF16),
            ("d_w1", [P, 2 * K], BF16),
            ("d_q0", [P, ROW_PAD], BF16),
            ("d_q1", [P, ROW_PAD], BF16),
            ("d_g0", [P, K * ROW_PAD], BF16),
            ("d_g1", [P, K * ROW_PAD], BF16),
        ]:
            dbg[nm] = nc.declare_dram_parameter(nm, shape, dt, isOutput=True)

    from contextlib import ExitStack

    with ExitStack() as ctx:
        e = ctx.enter_context

        idx_sb = e(nc.sbuf_tensor("idx_sb", [P, n_tiles * IDX_COLS], I16))
        # pre-pass: 2-chunk rotating bf16 staging + bulk norm buffers
        XCOL = 2 * C
        TPC = 8
        nch = max(1, n_tiles // TPC)
        tab_sb = e(nc.sbuf_tensor("tab_sb", [P, n_tiles * XCOL], BF16))
        nn_all = e(nc.sbuf_tensor("nn_all", [P, 2 * n_tiles], FP32))
        nrec_all = e(nc.sbuf_tensor("nrec_all", [P, 2 * n_tiles], FP32))
        ninv_sb = e(nc.sbuf_tensor("ninv_sb", [P, 2 * n_tiles], FP32))
        ninv_bf = e(nc.sbuf_tensor("ninv_bf", [P, 2 * n_tiles], BF16))
        # main-loop buffers (G/Q x8 so gathers stream ahead; small ones x2)
        G = [e(nc.sbuf_tensor(f"G{i}", [P, K * ROW_PAD], BF16)) for i in range(8)]
        scr = e(nc.sbuf_tensor("scr", [P, C], BF16))
        dots = [e(nc.sbuf_tensor(f"dots{i}", [P, 2 * K], FP32)) for i in range(2)]
        sim2 = [e(nc.sbuf_tensor(f"sim2_{i}", [P, 2 * K], FP32)) for i in range(2)]
        ee = [e(nc.sbuf_tensor(f"e{i}", [P, 2 * K], FP32)) for i in range(2)]
        ss = [e(nc.sbuf_tensor(f"s{i}_", [P, 2], FP32)) for i in range(2)]
        sinv = e(nc.sbuf_tensor("sinv", [P, 2], FP32))
        dscr = e(nc.sbuf_tensor("dscr", [P, 2], FP32))
        ww = [e(nc.sbuf_tensor(f"w{i}", [P, 2 * K], BF16)) for i in range(4)]
        out_sb = e(nc.sbuf_tensor("out_sb", [33, C], FP32))
        acc = e(nc.psum_tensor("acc", [64, C], FP32))

        xload = e(nc.semaphore("xload"))
        vchk = e(nc.semaphore("vchk"))
        vpre = e(nc.semaphore("vpre"))
        spre = e(nc.semaphore("spre"))
        tdone = e(nc.semaphore("tdone"))
        isem = e(nc.semaphore("isem"))
        gsem = [e(nc.semaphore(f"gsem{i}")) for i in range(8)]
        v1 = e(nc.semaphore("v1"))
        v2 = e(nc.semaphore("v2"))
        aexp = e(nc.semaphore("aexp"))
        pe_done = e(nc.semaphore("pe_done"))
        vfin = e(nc.semaphore("vfin"))
        osem = e(nc.semaphore("osem"))

        block = e(nc.Block())
        n_main = main_reps * n_tiles
        stores_per_rep = nch + 1
        n_stores = pre_reps * stores_per_rep

        def k3(ap):  # [P, 2K] -> [P, K, 2] (k-major pairs)
            return ap.rearrange("p (a b) -> p a b", b=2)

        def kT(ap):  # [P, 2K] -> [P, 2, 9] transposed view (reduce over k)
            return ap.rearrange("p (a b) -> p b a", b=2)

        def bcast2(ap2):  # [P, 2] -> [P, K, 2] with step-0 broadcast over K
            return ap2.rearrange("p (o b) -> p o b", o=1).to_broadcast([P, K, 2])

        @block.sync
        def _(sync: bass.BassEngine):
            # ---- pre-pass: store x-part chunks + the norm tail ----
            XC = XCOL
            for r in range(pre_reps):
                for c in range(nch):
                    sync.wait_ge(xload, 32 * (r * nch + c + 1))
                    rows = slice(c * TPC * P, (c + 1) * TPC * P)
                    sync.dma_start(
                        out=table[rows, 0:XC].rearrange("(g p) r -> p g r", p=P),
                        in_=tab_sb[
                            :, c * TPC * XC : (c + 1) * TPC * XC
                        ].rearrange("p (g r) -> p g r", r=XC),
                    ).then_inc(tdone, 16)
                sync.wait_ge(spre, r + 1)
                sync.dma_start(
                    out=table[:, XC : XC + 2].rearrange("(g p) r -> p g r", p=P),
                    in_=ninv_bf[:].rearrange("p (g b) -> p g b", b=2),
                ).then_inc(tdone, 16)
            # ---- epilogue (queries come straight from tab_sb) ----
            sync.wait_ge(vfin, 1)
            sync.dma_start(out=out_ext[0:1, :], in_=out_sb[0:1, :]).then_inc(osem, 16)
            sync.dma_start(out=out_ext[1:2, :], in_=out_sb[32:33, :]).then_inc(
                osem, 16
            )
            n_os = 32
            if debug:
                for nm, buf in []:
                    sync.dma_start(out=dbg[nm][:], in_=buf[:]).then_inc(osem, 16)
                    n_os += 16
            sync.wait_ge(osem, n_os)

        @block.vector
        def _(vector: bass.BassVectorEngine):
            # ---- pre-pass: bulk query norms from the staged bf16 chunks ----
            for r in range(pre_reps):
                for c in range(nch):
                    vector.wait_ge(xload, 32 * (r * nch + c + 1))
                    for tt in range(TPC):
                        t = c * TPC + tt
                        for b in range(2):
                            st = vector.scalar_tensor_tensor(
                                out=scr[:],
                                in0=tab_sb[
                                    :, t * XCOL + b * C : t * XCOL + (b + 1) * C
                                ],
                                scalar=1.0,
                                in1=tab_sb[
                                    :, t * XCOL + b * C : t * XCOL + (b + 1) * C
                                ],
                                op0=OP.mult,
                                op1=OP.mult,
                                accum_out=nn_all[:, 2 * t + b : 2 * t + b + 1],
                            )
                    st.then_inc(vchk, 1)
                for _ in range(3):
                    vector.tensor_copy(out=scr[:], in_=tab_sb[:, 0:C])
                vector.reciprocal(out=nrec_all[:], in_=nn_all[:]).then_inc(vpre, 1)

            # ---- main loop ----
            # iteration t: [A1] recip for t-2, [C] dots for t, [B] sim2 for
            # t-1, [A2] weights for t-2.  Reduction-class outputs (STT
            # accum_out, ACT accum_out, reciprocal) land late on HW, so every
            # such value is read >= ~19 instructions after it is produced.
            def phase_a1(m):  # sinv = 1/ss for tile m (ss from ACT accum)
                vector.wait_ge(aexp, m + 1)
                vector.reciprocal(out=sinv[:], in_=ss[m % 2][:])

            def phase_b(m):  # sim2 for tile m (reads dots[m%2], late-landing)
                gv = G[m % 8][:].rearrange("p (g r) -> p g r", r=ROW_PAD)
                vector.tensor_tensor(
                    out=k3(sim2[m % 2][:]),
                    in0=k3(dots[m % 2][:]),
                    in1=gv[:, :, 2 * C : 2 * C + 2],
                    op=OP.mult,
                ).then_inc(v1, 1)

            def phase_a2(m):  # ww = ee * sinv for tile m
                if m >= 4:
                    vector.wait_ge(pe_done, m - 3)  # ww[m%4] free
                vector.tensor_tensor(
                    out=k3(ww[m % 4][:]),
                    in0=k3(ee[m % 2][:]),
                    in1=bcast2(sinv[:]),
                    op=OP.mult,
                ).then_inc(v2, 1)

            def spacer(n=2):
                for _ in range(n):
                    vector.tensor_copy(out=scr[:], in_=tab_sb[:, 0:C])

            vector.wait_ge(spre, pre_reps)  # ninv_sb resident
            for g in range(n_main):
                t = g % n_tiles
                if g >= 3:
                    phase_a1(g - 3)
                vector.wait_ge(gsem[g % 8], 32 * (g // 8 + 1))
                for k in range(1 if "stt" in ablate else K):
                    for b in range(2):
                        vector.scalar_tensor_tensor(
                            out=scr[:],
                            in0=tab_sb[
                                :, t * XCOL + b * C : t * XCOL + (b + 1) * C
                            ],
                            scalar=ninv_sb[:, 2 * t + b : 2 * t + b + 1],
                            in1=G[g % 8][
                                :, k * ROW_PAD + b * C : k * ROW_PAD + (b + 1) * C
                            ],
                            op0=OP.mult,
                            op1=OP.mult,
                            accum_out=dots[g % 2][:, k * 2 + b : k * 2 + b + 1],
                        )
                if g >= 1:
                    phase_b(g - 1)
                if g >= 3:
                    phase_a2(g - 3)
            # tail (in-loop phases covered m <= n_main-4)
            for m in (n_main - 3, n_main - 2, n_main - 1):
                if m < 0 or m <= n_main - 4:
                    continue
                if m == n_main - 1:
                    spacer()
                    phase_b(m)
                phase_a1(m)
                spacer(3)
                phase_a2(m)
            # epilogue: PSUM -> SBUF with 1/T scaling
            vector.wait_ge(pe_done, n_main)
            vector.tensor_scalar_mul(
                out=out_sb[0:1, :], in0=acc[0:1, :], scalar1=1.0 / (t_loc * main_reps)
            )
            vector.tensor_scalar_mul(
                out=out_sb[32:33, :], in0=acc[32:33, :], scalar1=1.0 / (t_loc * main_reps)
            ).then_inc(vfin, 1)

        @block.scalar
        def _(scalar: bass.BassScalarEngine):
            # ---- pre-pass: ninv = sqrt(1/nn) -> resident f32 + bf16 copy ----
            for r in range(pre_reps):
                scalar.wait_ge(vpre, r + 1)
                scalar.activation(out=ninv_sb[:], in_=nrec_all[:], func=AF.Sqrt)
                scalar.activation(
                    out=ninv_bf[:], in_=ninv_sb[:], func=AF.Copy
                ).then_inc(spre, 1)
            # ---- main loop: exp with fused per-batch denominators; the
            # aexp inc rides a trailing dummy op so the accum lands first ----
            for g in range(n_main):
                scalar.wait_ge(v1, g + 1)
                if g >= 2:
                    scalar.wait_ge(v2, g - 1)  # ee/ss[g%2] free (a2 lag 3)
                s23 = k3(sim2[g % 2][:])
                e3 = k3(ee[g % 2][:])
                for b in range(2):
                    scalar.activation(
                        out=e3[:, :, b : b + 1],
                        in_=s23[:, :, b : b + 1],
                        func=AF.Exp,
                        accum_out=ss[g % 2][:, b : b + 1],
                    )
                scalar.activation(
                    out=dscr[:], in_=sim2[g % 2][:, 0:2], func=AF.Copy
                ).then_inc(aexp, 1)

        @block.tensor
        def _(tensor: bass.BassTensorEngine):
            for g in range(n_main):
                tensor.wait_ge(v2, g + 1)
                for k in range(1 if "pe" in ablate else K):
                    for b in range(2):
                        mm = tensor.matmul(
                            out=acc[32 * b : 32 * b + 1, :],
                            lhsT=ww[g % 4][:, k * 2 + b : k * 2 + b + 1],
                            rhs=G[g % 8][
                                :, k * ROW_PAD + b * C : k * ROW_PAD + (b + 1) * C
                            ],
                            start=(g == 0 and k == 0),
                            stop=(g == n_main - 1 and k == K - 1),
                            skip_group_check=True,
                        )
                mm.then_inc(pe_done, 1)

        @block.gpsimd
        def _(gpsimd: bass.BassGpSimd):
            gpsimd.load_library(mlp)
            gpsimd.dma_start(out=idx_sb[:], in_=idx_ext[:]).then_inc(isem, 16)
            # ---- pre-pass: cast-DMA X f32 -> staged bf16, chunked ----
            for r in range(pre_reps):
                for c in range(nch):
                    if r > 0:
                        gpsimd.wait_ge(vchk, (r - 1) * nch + c + 1)
                        gpsimd.wait_ge(
                            tdone, 16 * ((r - 1) * stores_per_rep + c + 1)
                        )
                    rows = slice(c * TPC * P, (c + 1) * TPC * P)
                    tv = tab_sb[
                        :, c * TPC * XCOL : (c + 1) * TPC * XCOL
                    ].rearrange("p (g b c) -> p g b c", b=2, c=C)
                    for b in range(2):
                        gpsimd.dma_start(
                            out=tv[:, :, b, :],
                            in_=x_ext[b, rows, :].rearrange("(g p) c -> p g c", p=P),
                        ).then_inc(xload, 16)
            gpsimd.wait_ge(isem, 16)
            gpsimd.wait_ge(tdone, 16 * n_stores)  # full table resident
            for g in range(n_main):
                t = g % n_tiles
                if g >= 8:
                    gpsimd.wait_ge(pe_done, g - 7)  # G[g%8] free
                gv3 = G[g % 8][:].rearrange("p (g r) -> p g r", r=ROW_PAD)
                # 1024+128 split: the large first call keeps the SDMA
                # stream busier across call boundaries (~20% faster than 5+4)
                spans = ((0, 1), (1, 2)) if "gather" in ablate else ((0, 8), (8, K))
                sp_flag = True
                for g0, g1 in spans:
                    n = (g1 - g0) * P
                    if "halfbytes" in ablate:
                        gpsimd.dma_gather(
                            ghalf[:].rearrange("p (g r) -> p g r", r=512)[
                                :, g0:g1, :
                            ],
                            table[:, 0:512],
                            idx_sb[
                                :,
                                t * IDX_COLS + g0 * 8 : t * IDX_COLS + g1 * 8,
                            ],
                            n,
                            n,
                            512,
                            elem_step=ROW_PAD,
                        ).then_inc(gsem[g % 8], 16)
                    else:
                        gpsimd.dma_gather(
                            gv3[:, g0:g1, :],
                            table[:],
                            idx_sb[
                                :,
                                t * IDX_COLS + g0 * 8 : t * IDX_COLS + g1 * 8,
                            ],
                            n,
                            n,
                            ROW_PAD,
                            single_packet=sp_flag,
                        ).then_inc(gsem[g % 8], 16)

    nc.compile()
    return nc


def make_idx_table(neighbor_idx: np.ndarray, n_tiles: int = NT) -> np.ndarray:
    """Host-side index preprocessing into dma_gather's wrapped int16 layout.

    Flat order per tile: i = k*128 + p  ->  neighbor_idx[t0+p, k].
    Wrapped: idx_sb[q, tile*IDX_COLS + c] = flat[c*16 + q%16].
    """
    nb = np.asarray(neighbor_idx).astype(np.int16)  # values < 4096
    cols = np.empty((P, n_tiles * IDX_COLS), dtype=np.int16)
    for t in range(n_tiles):
        flat = nb[t * P : (t + 1) * P, :].T.reshape(-1)  # [K*P], k-major
        wrap = flat.reshape(IDX_COLS, 16).T  # [16, IDX_COLS]
        cols[:, t * IDX_COLS : (t + 1) * IDX_COLS] = np.tile(wrap, (8, 1))
    return cols


_NC_CACHE: dict = {}


def _get_nc():
    if "nc" not in _NC_CACHE:
        _NC_CACHE["nc"] = build_kernel()
    return _NC_CACHE["nc"]


def kernel(X: np.ndarray, neighbor_idx: np.ndarray, **_ignored) -> np.ndarray:
    X = np.asarray(X, dtype=np.float32)
    idx_cols = make_idx_table(neighbor_idx)
    nc = _get_nc()
    core_ids = list(range(N_CORES))
    in_maps = [
        {"X": np.ascontiguousarray(X[i * B_LOC : (i + 1) * B_LOC]), "idx": idx_cols}
        for i in core_ids
    ]
    res = run_bass_kernel_spmd(nc, in_maps, core_ids)
    outs = [res.results[i]["out"] for i in range(N_CORES)]  # each [B_LOC, C]
    full = np.concatenate(outs, axis=0).reshape(B, 1, C).astype(np.float32)
    return full


if __name__ == "__main__":
    rng = np.random.default_rng(0)
    X = rng.standard_normal((B, T, C), dtype=np.float32)
    nb = rng.integers(0, T, size=(T, K)).astype(np.int64)
    out = kernel(X, nb)
    print("out", out.shape, out.dtype, float(np.abs(out).mean()))

